# revision 1
# baseline (speedup 1.0000x reference)
"""Trainium2 Bass kernel for a segmented tensor-product contraction.

Computation (per batch row z, channel u, segments of width U=128):
  out[z, so, u] += c_p * x0[i0[z], s0_p, u] * prod_k x1[z, sk_p, u]
for 256 paths of degree 1..3.

Strategy:
  - Data-parallel over z across 8 NeuronCores (512 rows each).
  - On-chip layout: [u (partitions) x z (free dim)] per segment, so every
    path op is a [128, 512] elementwise instruction with big free dim.
  - x0 row gather: host builds a one-hot(i0) matrix per core; the
    TensorEngine computes x0gT[s] = x0[:, s]^T @ onehot, which lands the
    gathered table directly in [u, z] layout (gather + transpose for free).
  - Path products/accumulation: DVE tensor_tensor / scalar_tensor_tensor
    ops with the per-path coefficient folded into fused scalar slots.
  - Host transposes x1 into [s*u, z] layout per core and transposes the
    output back (pure layout transforms; all FLOPs happen on device).
"""

import os

import numpy as np

U = 128
S = 16
NELEM = 64
Z = 4096
NCORES = 8
ZS = Z // NCORES  # 512 rows per core

# set by the last kernel() call when profiling is enabled (BASS_TRACE=1)
LAST_EXEC_NS = None
LAST_RESULTS = None

F32 = "float32"


def _build_plan(idxs, coeffs):
    """Build the elementwise op schedule from path index arrays.

    Returns a list of op tuples executed per core:
      ('pair', dst_key, a, b)                   dst = x1t[a] * x1t[b]
      ('stt', dst, in0, scalar, in1, op0, op1)  dst = (in0 op0 s) op1 in1
      ('tt', dst, in0, in1, op)
      ('ts', dst, in0, scalar)                  dst = in0 * s
    Tensor refs are ('x1', s), ('x0g', s), ('out', so), ('pair', key),
    ('tmp', id). Ops are ordered so shared pair products are built once
    and consumed immediately.
    """
    paths = []  # (degree, x1segs, s0, so, coeff)
    for idx, cf in zip(idxs, coeffs):
        d = idx.shape[1] - 2
        for r, c in zip(idx, cf):
            r = [int(v) for v in r]
            paths.append((d, tuple(r[:d]), r[d], r[d + 1], float(c)))

    # group degree-3 paths by unique triple
    from collections import defaultdict

    tri_users = defaultdict(list)  # triple -> [path]
    pair_users = defaultdict(lambda: {"d2": [], "tri": []})
    d1_paths = []
    for p in paths:
        d, segs, s0, so, c = p
        if d == 1:
            d1_paths.append(p)
        elif d == 2:
            pair_users[segs]["d2"].append(p)
        else:
            tri_users[segs].append(p)
    for t in tri_users:
        pair_users[(t[0], t[1])]["tri"].append(t)

    ops = []
    out_started = set()  # so values whose accumulator has been written
    tmp_id = [0]

    def new_tmp():
        tmp_id[0] += 1
        return ("tmp", tmp_id[0])

    def emit_term(term_ref, so):
        """Accumulate a finished path term (a tmp/holding tile) into out."""
        # term_ref written by the caller's op with dst chosen by this fn
        raise NotImplementedError

    def accum(make_term, so):
        """make_term(dst) emits op(s) writing the path term to dst."""
        if so not in out_started:
            make_term(("out", so))
            out_started.add(so)
        else:
            t = new_tmp()
            make_term(t)
            ops.append(("tt", ("out", so), ("out", so), t, "add"))

    # degree-1 paths first (cheap, start many out accumulators)
    for d, segs, s0, so, c in d1_paths:
        s1 = segs[0]
        accum(
            lambda dst, s1=s1, c=c, s0=s0: ops.append(
                ("stt", dst, ("x1", s1), c, ("x0g", s0), "mult", "mult")
            ),
            so,
        )

    # pair-major: build each pair, then its d2 paths and d3 triples
    for (a, b), users in pair_users.items():
        pk = ("pair", (a, b))
        ops.append(("tt", pk, ("x1", a), ("x1", b), "mult"))
        for d, segs, s0, so, c in users["d2"]:
            accum(
                lambda dst, pk=pk, c=c, s0=s0: ops.append(
                    ("stt", dst, pk, c, ("x0g", s0), "mult", "mult")
                ),
                so,
            )
        for t in users["tri"]:
            s3 = t[2]
            tpaths = tri_users[t]
            if len(tpaths) == 1:
                d, segs, s0, so, c = tpaths[0]
                # fold coeff into the triple build
                trk = new_tmp()
                ops.append(("stt", trk, pk, c, ("x1", s3), "mult", "mult"))
                accum(
                    lambda dst, trk=trk, s0=s0: ops.append(
                        ("tt", dst, trk, ("x0g", s0), "mult")
                    ),
                    so,
                )
            else:
                trk = new_tmp()
                ops.append(("tt", trk, pk, ("x1", s3), "mult"))
                for d, segs, s0, so, c in tpaths:
                    accum(
                        lambda dst, trk=trk, c=c, s0=s0: ops.append(
                            ("stt", dst, trk, c, ("x0g", s0), "mult", "mult")
                        ),
                        so,
                    )
    assert len(out_started) == S or True
    return ops, sorted(out_started)


def _build_bass(ops, out_started, dtype_name):
    """Construct and compile the Bass program. Returns nc."""
    import concourse.bacc as bacc
    import concourse.mybir as mybir
    from concourse.tile import TileContext

    dt = mybir.dt.float32 if dtype_name == F32 else mybir.dt.bfloat16
    alu = {
        "mult": mybir.AluOpType.mult,
        "add": mybir.AluOpType.add,
    }

    nc = bacc.Bacc("TRN2", debug=False)

    x1t_d = nc.dram_tensor("x1t", [S * U, ZS], dt, kind="ExternalInput")
    x0_d = nc.dram_tensor("x0w", [NELEM, S * U], dt, kind="ExternalInput")
    oh_d = nc.dram_tensor("oh", [NELEM, ZS], dt, kind="ExternalInput")
    out_d = nc.dram_tensor("outt", [S * U, ZS], dt, kind="ExternalOutput")

    with TileContext(nc) as tc:
        with tc.tile_pool(name="persist", bufs=1) as persist, tc.tile_pool(
            name="psum", bufs=4, space="PSUM"
        ) as psum_pool, tc.tile_pool(name="scratch", bufs=6) as scratch:
            # persistent tiles
            x1t = persist.tile([U, S * ZS], dt, tag="x1t")
            x0g = persist.tile([U, S * ZS], dt, tag="x0g")
            out_sb = persist.tile([U, S * ZS], dt, tag="out")
            x0_sb = persist.tile([NELEM, S * U], dt, tag="x0w")
            oh_sb = persist.tile([NELEM, ZS], dt, tag="oh")

            def seg(t, s):
                return t[:, s * ZS : (s + 1) * ZS]

            # load inputs
            for s in range(S):
                nc.sync.dma_start(
                    out=seg(x1t, s), in_=x1t_d[s * U : (s + 1) * U, :]
                )
            nc.sync.dma_start(out=x0_sb[:], in_=x0_d[:])
            nc.sync.dma_start(out=oh_sb[:], in_=oh_d[:])

            # gather matmuls: x0g[:, s] = x0[:, s*U:(s+1)*U]^T @ onehot
            for s in range(S):
                pt = psum_pool.tile([U, ZS], mybir.dt.float32, tag="gps")
                nc.tensor.matmul(
                    pt[:],
                    x0_sb[:, s * U : (s + 1) * U],
                    oh_sb[:],
                    start=True,
                    stop=True,
                )
                nc.scalar.copy(out=seg(x0g, s), in_=pt[:])

            # elementwise op schedule
            live = {}  # key -> AP for pair/tmp tiles

            def ref(r):
                kind = r[0]
                if kind == "x1":
                    return seg(x1t, r[1])
                if kind == "x0g":
                    return seg(x0g, r[1])
                if kind == "out":
                    return seg(out_sb, r[1])
                return live[r]

            def dst_ap(r):
                if r[0] in ("pair", "tmp"):
                    t = scratch.tile([U, ZS], dt, tag="scr")
                    live[r] = t
                    return t[:]
                return ref(r)

            for op in ops:
                if op[0] == "tt":
                    _, d, a, b, o = op
                    nc.vector.tensor_tensor(
                        out=dst_ap(d), in0=ref(a), in1=ref(b), op=alu[o]
                    )
                elif op[0] == "stt":
                    _, d, a, sc, b, o0, o1 = op
                    nc.vector.scalar_tensor_tensor(
                        out=dst_ap(d),
                        in0=ref(a),
                        scalar=sc,
                        in1=ref(b),
                        op0=alu[o0],
                        op1=alu[o1],
                    )
                elif op[0] == "ts":
                    _, d, a, sc = op
                    nc.vector.tensor_scalar(
                        out=dst_ap(d),
                        in0=ref(a),
                        scalar1=sc,
                        scalar2=None,
                        op0=alu["mult"],
                    )
                else:
                    raise ValueError(op)

            # zero any never-written output segments, then store
            for s in range(S):
                if s not in out_started:
                    nc.vector.memset(seg(out_sb, s), 0.0)
                nc.sync.dma_start(
                    out=out_d[s * U : (s + 1) * U, :], in_=seg(out_sb, s)
                )

    nc.compile()
    return nc


def kernel(x0, x1, coeff1, coeff2, coeff3, i0, idx1, idx2, idx3):
    global LAST_EXEC_NS, LAST_RESULTS
    from concourse.bass_utils import run_bass_kernel_spmd

    x0 = np.asarray(x0, dtype=np.float32)
    x1 = np.asarray(x1, dtype=np.float32)
    i0 = np.asarray(i0).astype(np.int64)
    idxs = [np.asarray(a) for a in (idx1, idx2, idx3)]
    coeffs = [np.asarray(c, dtype=np.float32) for c in (coeff1, coeff2, coeff3)]

    dtype_name = os.environ.get("KERNEL_DTYPE", F32)
    npdt = np.float32
    if dtype_name != F32:
        import ml_dtypes

        npdt = ml_dtypes.bfloat16

    ops, out_started = _build_plan(idxs, coeffs)
    nc = _build_bass(ops, out_started, dtype_name)

    # host-side sharding + layout transforms
    in_maps = []
    eye = np.arange(NELEM)
    for c in range(NCORES):
        zl, zh = c * ZS, (c + 1) * ZS
        shard = x1[zl:zh]  # [ZS, S*U]
        x1t = np.ascontiguousarray(
            shard.reshape(ZS, S, U).transpose(1, 2, 0).reshape(S * U, ZS)
        ).astype(npdt)
        oh = (i0[zl:zh][None, :] == eye[:, None]).astype(npdt)
        in_maps.append(
            {"x1t": x1t, "x0w": x0.astype(npdt), "oh": oh}
        )

    trace = os.environ.get("BASS_TRACE", "") not in ("", "0")
    res = run_bass_kernel_spmd(
        nc, in_maps, core_ids=list(range(NCORES)), trace=trace
    )
    LAST_EXEC_NS = res.exec_time_ns
    LAST_RESULTS = res

    out = np.empty((Z, S * U), dtype=np.float32)
    for c in range(NCORES):
        outt = np.asarray(res.results[c]["outt"], dtype=np.float32)
        out[c * ZS : (c + 1) * ZS] = (
            outt.reshape(S, U, ZS).transpose(2, 0, 1).reshape(ZS, S * U)
        )
    return out


# revision 6
# speedup vs baseline: 1.2797x; 1.2797x over previous
"""Trainium2 Bass kernel for a segmented tensor-product contraction.

Computation (per batch row z, channel u, segments of width U=128):
  out[z, so, u] += c_p * x0[i0[z], s0_p, u] * prod_k x1[z, sk_p, u]
for 256 paths of degree 1..3 over S=16 segments.

Strategy:
  - Data-parallel over z across 8 NeuronCores (512 rows each).
  - On-chip layout: [u (partitions) x z (free dim)] per segment, so every
    path op is a [128, 512] elementwise instruction with a big free dim.
  - x0 row gather: host builds one-hot(i0) per core; TensorEngine computes
    x0gT[s] = x0[:, s]^T @ onehot - the gathered table lands directly in
    [u, z] layout (gather + transpose for free).
  - Factorization: sg(s0,s) = x0g[s0] * x1[s] suffix products shared by
    all path degrees; d3 prefix pairs pr(a,b) = x1[a] * x1[b]. Each path
    is then ONE fused scalar_tensor_tensor op (coefficient folded in).
  - Output accumulation on the TensorEngine: identity-matmul each path
    term into a per-segment PSUM bank (exact f32 adds, no DVE add ops).
    16 output segments -> two groups of 8 banks.
  - A tunable fraction of product builds runs on GpSimd (tensor_tensor
    there never contends with DVE's ports).
"""

import os
from collections import defaultdict

import numpy as np

U = 128
S = 16
NELEM = 64
Z = 4096
NCORES = 8
ZS = Z // NCORES  # 512 rows per core

LAST_EXEC_NS = None
LAST_RESULTS = None

F32 = "float32"


def _build_plan(idxs, coeffs):
    """Build the per-so-group op schedule.

    Returns (groups, so_list) where groups is a list (one per so-group) of
    op tuples:
      ('pair', (a,b))            pr = x1[a] * x1[b]
      ('sg', (s0,s))             sg = x0g[s0] * x1[s]
      ('path', kind, ref, c, ref2, so)   tmp = (ref * c) * ref2 -> accum so
    refs: ('x1',s) ('sg',(s0,s)) ('pair',(a,b))
    """
    paths = []  # (degree, x1segs, s0, so, coeff)
    for idx, cf in zip(idxs, coeffs):
        d = idx.shape[1] - 2
        for r, c in zip(idx, cf):
            r = [int(v) for v in r]
            paths.append((d, tuple(r[:d]), r[d], r[d + 1], float(c)))

    so_groups = [list(range(8)), list(range(8, 16))]
    groups = []
    for sos in so_groups:
        gpaths = [p for p in paths if p[3] in sos]
        ops = []
        have_sg = set()
        have_pair = set()

        def need_sg(s0, s):
            if (s0, s) not in have_sg:
                have_sg.add((s0, s))
                ops.append(("sg", (s0, s)))

        def need_pair(a, b):
            if (a, b) not in have_pair:
                have_pair.add((a, b))
                ops.append(("pair", (a, b)))

        # degree 1 and 2 first (only sg products), then d3 by prefix pair
        for d, segs, s0, so, c in gpaths:
            if d == 1:
                need_sg(s0, segs[0])
                ops.append(("path", "d1", ("sg", (s0, segs[0])), c, None, so))
            elif d == 2:
                need_sg(s0, segs[1])
                ops.append(
                    ("path", "d2", ("x1", segs[0]), c, ("sg", (s0, segs[1])), so)
                )
        d3 = [p for p in gpaths if p[0] == 3]
        by_pair = defaultdict(list)
        for p in d3:
            by_pair[(p[1][0], p[1][1])].append(p)
        for (a, b), plist in by_pair.items():
            need_pair(a, b)
            for d, segs, s0, so, c in plist:
                need_sg(s0, segs[2])
                ops.append(
                    ("path", "d3", ("pair", (a, b)), c, ("sg", (s0, segs[2])), so)
                )
        groups.append((sos, ops))
    return groups


def _build_bass(groups, dtype_name, pool_frac):
    """Construct and compile the Bass program. Returns nc."""
    import concourse.bacc as bacc
    import concourse.mybir as mybir
    from concourse.tile import TileContext

    dt = mybir.dt.float32 if dtype_name == F32 else mybir.dt.bfloat16
    MULT = mybir.AluOpType.mult

    nc = bacc.Bacc("TRN2", debug=False)

    x1t_d = nc.dram_tensor("x1t", [S * U, ZS], dt, kind="ExternalInput")
    x0_d = nc.dram_tensor("x0w", [NELEM, S * U], dt, kind="ExternalInput")
    oh_d = nc.dram_tensor("oh", [NELEM, ZS], dt, kind="ExternalInput")
    id_d = nc.dram_tensor("ident", [U, U], dt, kind="ExternalInput")
    out_d = nc.dram_tensor("outt", [S * U, ZS], dt, kind="ExternalOutput")

    # count max live sg tiles per group for pool sizing
    max_sg = max(
        sum(1 for op in ops if op[0] == "sg") for _, ops in groups
    )

    with TileContext(nc) as tc:
        with tc.tile_pool(name="persist", bufs=1) as persist, tc.tile_pool(
            name="sg", bufs=max_sg + 1
        ) as sg_pool, tc.tile_pool(name="pair", bufs=6) as pair_pool, tc.tile_pool(
            name="tmp", bufs=10
        ) as tmp_pool:
            # persistent tiles
            x1t = persist.tile([U, S * ZS], dt, tag="x1t")
            x0g = persist.tile([U, S * ZS], dt, tag="x0g")
            out_sb = persist.tile([U, S * ZS], dt, tag="out")
            x0_sb = persist.tile([NELEM, S * U], dt, tag="x0w")
            oh_sb = persist.tile([NELEM, ZS], dt, tag="oh")
            id_sb = persist.tile([U, U], dt, tag="ident")

            def seg(t, s):
                return t[:, s * ZS : (s + 1) * ZS]

            # load inputs
            for s in range(S):
                nc.sync.dma_start(out=seg(x1t, s), in_=x1t_d[s * U : (s + 1) * U, :])
            nc.sync.dma_start(out=x0_sb[:], in_=x0_d[:])
            nc.sync.dma_start(out=oh_sb[:], in_=oh_d[:])
            nc.sync.dma_start(out=id_sb[:], in_=id_d[:])

            # gather matmuls: x0g[:, s] = x0[:, s*U:(s+1)*U]^T @ onehot
            with tc.tile_pool(name="gpsum", bufs=4, space="PSUM") as gpsum:
                for s in range(S):
                    pt = gpsum.tile([U, ZS], mybir.dt.float32, tag="gps")
                    nc.tensor.matmul(
                        pt[:],
                        x0_sb[:, s * U : (s + 1) * U],
                        oh_sb[:],
                        start=True,
                        stop=True,
                    )
                    nc.scalar.copy(out=seg(x0g, s), in_=pt[:])

            # elementwise schedule, two so-groups
            build_i = [0]

            def build_engine():
                # route a fraction of product builds to GpSimd
                build_i[0] += 1
                if pool_frac > 0 and (build_i[0] % 100) < pool_frac * 100:
                    return nc.gpsimd
                return nc.vector

            for sos, ops in groups:
                live = {}
                # last path index per so within this group (for stop flag)
                path_ops = [op for op in ops if op[0] == "path"]
                last_for_so = {}
                first_for_so = {}
                for i, op in enumerate(path_ops):
                    so = op[5]
                    if so not in first_for_so:
                        first_for_so[so] = i
                    last_for_so[so] = i

                acc = {}  # so -> psum tile
                with tc.tile_pool(
                    name=f"acc{sos[0]}", bufs=8, space="PSUM"
                ) as acc_pool:
                    for so in sos:
                        if any(op[5] == so for op in path_ops):
                            acc[so] = acc_pool.tile(
                                [U, ZS], mybir.dt.float32, tag=f"acc{so % 8}",
                                name=f"acc_{so}", bufs=1,
                            )

                    def ref(r):
                        kind, key = r
                        if kind == "x1":
                            return seg(x1t, key)
                        return live[r]

                    pi = 0
                    for op in ops:
                        if op[0] == "sg":
                            s0, s = op[1]
                            t = sg_pool.tile([U, ZS], dt, tag="sg")
                            live[("sg", (s0, s))] = t
                            build_engine().tensor_tensor(
                                out=t[:], in0=seg(x0g, s0), in1=seg(x1t, s), op=MULT
                            )
                        elif op[0] == "pair":
                            a, b = op[1]
                            t = pair_pool.tile([U, ZS], dt, tag="pair")
                            live[("pair", (a, b))] = t
                            build_engine().tensor_tensor(
                                out=t[:], in0=seg(x1t, a), in1=seg(x1t, b), op=MULT
                            )
                        else:
                            _, kind, r1, c, r2, so = op
                            t = tmp_pool.tile([U, ZS], dt, tag="tmp")
                            if kind == "d1":
                                nc.vector.tensor_scalar(
                                    out=t[:],
                                    in0=ref(r1),
                                    scalar1=c,
                                    scalar2=None,
                                    op0=MULT,
                                )
                            else:
                                nc.vector.scalar_tensor_tensor(
                                    out=t[:],
                                    in0=ref(r1),
                                    scalar=c,
                                    in1=ref(r2),
                                    op0=MULT,
                                    op1=MULT,
                                )
                            nc.tensor.matmul(
                                acc[so][:],
                                id_sb[:],
                                t[:],
                                start=(pi == first_for_so[so]),
                                stop=(pi == last_for_so[so]),
                            )
                            pi += 1

                    # evacuate this group's accumulators
                    for so in sos:
                        if so in acc:
                            nc.scalar.copy(out=seg(out_sb, so), in_=acc[so][:])
                        else:
                            nc.vector.memset(seg(out_sb, so), 0.0)

            for s in range(S):
                nc.sync.dma_start(out=out_d[s * U : (s + 1) * U, :], in_=seg(out_sb, s))

    nc.compile()
    return nc


def kernel(x0, x1, coeff1, coeff2, coeff3, i0, idx1, idx2, idx3):
    global LAST_EXEC_NS, LAST_RESULTS
    from concourse.bass_utils import run_bass_kernel_spmd

    x0 = np.asarray(x0, dtype=np.float32)
    x1 = np.asarray(x1, dtype=np.float32)
    i0 = np.asarray(i0).astype(np.int64)
    idxs = [np.asarray(a) for a in (idx1, idx2, idx3)]
    coeffs = [np.asarray(c, dtype=np.float32) for c in (coeff1, coeff2, coeff3)]

    dtype_name = os.environ.get("KERNEL_DTYPE", "bfloat16")
    pool_frac = float(os.environ.get("KERNEL_POOL_FRAC", "0.0"))
    npdt = np.float32
    if dtype_name != F32:
        import ml_dtypes

        npdt = ml_dtypes.bfloat16

    groups = _build_plan(idxs, coeffs)
    nc = _build_bass(groups, dtype_name, pool_frac)

    # host-side sharding + layout transforms
    in_maps = []
    eye = np.arange(NELEM)
    ident = np.eye(U, dtype=npdt)
    x0c = x0.astype(npdt)
    for c in range(NCORES):
        zl, zh = c * ZS, (c + 1) * ZS
        shard = x1[zl:zh]  # [ZS, S*U]
        x1t = np.ascontiguousarray(
            shard.reshape(ZS, S, U).transpose(1, 2, 0).reshape(S * U, ZS)
        ).astype(npdt)
        oh = (i0[zl:zh][None, :] == eye[:, None]).astype(npdt)
        in_maps.append({"x1t": x1t, "x0w": x0c, "oh": oh, "ident": ident})

    trace = os.environ.get("BASS_TRACE", "") not in ("", "0")
    res = run_bass_kernel_spmd(nc, in_maps, core_ids=list(range(NCORES)), trace=trace)
    LAST_EXEC_NS = res.exec_time_ns
    LAST_RESULTS = res

    out = np.empty((Z, S * U), dtype=np.float32)
    for c in range(NCORES):
        outt = np.asarray(res.results[c]["outt"], dtype=np.float32)
        out[c * ZS : (c + 1) * ZS] = (
            outt.reshape(S, U, ZS).transpose(2, 0, 1).reshape(ZS, S * U)
        )
    return out


# revision 8
# speedup vs baseline: 1.3599x; 1.0627x over previous
"""Trainium2 Bass kernel for a segmented tensor-product contraction.

Computation (per batch row z, channel u, segments of width U=128):
  out[z, so, u] += c_p * x0[i0[z], s0_p, u] * prod_k x1[z, sk_p, u]
for 256 paths of degree 1..3 over S=16 segments.

Strategy:
  - Data-parallel over z across 8 NeuronCores (512 rows each).
  - On-chip layout: [u (partitions) x z (free dim)] per segment, so every
    path op is a [128, 512] elementwise instruction with a big free dim.
  - x0 row gather: host builds one-hot(i0) per core; TensorEngine computes
    x0gT[s] = x0[:, s]^T @ onehot - the gathered table lands directly in
    [u, z] layout (gather + transpose for free).
  - Factorization: sg(s0,s) = x0g[s0] * x1[s] suffix products shared by
    all path degrees; d3 prefix pairs pr(a,b) = x1[a] * x1[b]. Each path
    is then ONE fused scalar_tensor_tensor op (coefficient folded in).
  - Output accumulation on the TensorEngine: identity-matmul each path
    term into a per-segment PSUM bank (exact f32 adds, no DVE add ops).
    16 output segments -> two groups of 8 banks.
  - A tunable fraction of product builds runs on GpSimd (tensor_tensor
    there never contends with DVE's ports).
"""

import os
from collections import defaultdict

import numpy as np

U = 128
S = 16
NELEM = 64
Z = 4096
NCORES = 8
ZS = Z // NCORES  # 512 rows per core

LAST_EXEC_NS = None
LAST_RESULTS = None

F32 = "float32"


def _build_plan(idxs, coeffs):
    """Build the per-so-group op schedule.

    Returns (groups, so_list) where groups is a list (one per so-group) of
    op tuples:
      ('pair', (a,b))            pr = x1[a] * x1[b]
      ('sg', (s0,s))             sg = x0g[s0] * x1[s]
      ('path', kind, ref, c, ref2, so)   tmp = (ref * c) * ref2 -> accum so
    refs: ('x1',s) ('sg',(s0,s)) ('pair',(a,b))
    """
    paths = []  # (degree, x1segs, s0, so, coeff)
    for idx, cf in zip(idxs, coeffs):
        d = idx.shape[1] - 2
        for r, c in zip(idx, cf):
            r = [int(v) for v in r]
            paths.append((d, tuple(r[:d]), r[d], r[d + 1], float(c)))

    so_groups = [list(range(8)), list(range(8, 16))]
    groups = []
    for sos in so_groups:
        gpaths = [p for p in paths if p[3] in sos]
        ops = []
        have_sg = set()
        have_pair = set()

        def need_sg(s0, s):
            if (s0, s) not in have_sg:
                have_sg.add((s0, s))
                ops.append(("sg", (s0, s)))

        def need_pair(a, b):
            if (a, b) not in have_pair:
                have_pair.add((a, b))
                ops.append(("pair", (a, b)))

        # degree 1 and 2 first (only sg products), then d3 by prefix pair
        for d, segs, s0, so, c in gpaths:
            if d == 1:
                need_sg(s0, segs[0])
                ops.append(("path", "d1", ("sg", (s0, segs[0])), c, None, so))
            elif d == 2:
                need_sg(s0, segs[1])
                ops.append(
                    ("path", "d2", ("x1", segs[0]), c, ("sg", (s0, segs[1])), so)
                )
        d3 = [p for p in gpaths if p[0] == 3]
        by_pair = defaultdict(list)
        for p in d3:
            by_pair[(p[1][0], p[1][1])].append(p)
        for (a, b), plist in by_pair.items():
            need_pair(a, b)
            for d, segs, s0, so, c in plist:
                need_sg(s0, segs[2])
                ops.append(
                    ("path", "d3", ("pair", (a, b)), c, ("sg", (s0, segs[2])), so)
                )
        groups.append((sos, ops))
    return groups


def _build_bass(groups, dtype_name, pool_frac, act_frac=0.6):
    """Construct and compile the Bass program. Returns nc."""
    import concourse.bacc as bacc
    import concourse.mybir as mybir
    from concourse.tile import TileContext

    dt = mybir.dt.float32 if dtype_name == F32 else mybir.dt.bfloat16
    MULT = mybir.AluOpType.mult

    nc = bacc.Bacc("TRN2", debug=False)

    x1t_d = nc.dram_tensor("x1t", [S * U, ZS], dt, kind="ExternalInput")
    x0_d = nc.dram_tensor("x0w", [NELEM, S * U], dt, kind="ExternalInput")
    oh_d = nc.dram_tensor("oh", [NELEM, ZS], dt, kind="ExternalInput")
    id_d = nc.dram_tensor("ident", [U, U], dt, kind="ExternalInput")
    out_d = nc.dram_tensor("outt", [S * U, ZS], dt, kind="ExternalOutput")

    # count max live sg tiles per group for pool sizing
    max_sg = max(
        sum(1 for op in ops if op[0] == "sg") for _, ops in groups
    )

    with TileContext(nc) as tc:
        with tc.tile_pool(name="persist", bufs=1) as persist, tc.tile_pool(
            name="sg", bufs=max_sg + 1
        ) as sg_pool, tc.tile_pool(name="pair", bufs=6) as pair_pool, tc.tile_pool(
            name="tmp", bufs=10
        ) as tmp_pool:
            # persistent tiles
            x1t = persist.tile([U, S * ZS], dt, tag="x1t")
            x0g = persist.tile([U, S * ZS], dt, tag="x0g")
            out_sb = persist.tile([U, S * ZS], dt, tag="out")
            x0_sb = persist.tile([NELEM, S * U], dt, tag="x0w")
            oh_sb = persist.tile([NELEM, ZS], dt, tag="oh")
            id_sb = persist.tile([U, U], dt, tag="ident")

            def seg(t, s):
                return t[:, s * ZS : (s + 1) * ZS]

            # load inputs
            for s in range(S):
                nc.sync.dma_start(out=seg(x1t, s), in_=x1t_d[s * U : (s + 1) * U, :])
            nc.sync.dma_start(out=x0_sb[:], in_=x0_d[:])
            nc.sync.dma_start(out=oh_sb[:], in_=oh_d[:])
            nc.sync.dma_start(out=id_sb[:], in_=id_d[:])

            # gather matmuls: x0g[:, s] = x0[:, s*U:(s+1)*U]^T @ onehot
            with tc.tile_pool(name="gpsum", bufs=4, space="PSUM") as gpsum:
                for s in range(S):
                    pt = gpsum.tile([U, ZS], mybir.dt.float32, tag="gps")
                    nc.tensor.matmul(
                        pt[:],
                        x0_sb[:, s * U : (s + 1) * U],
                        oh_sb[:],
                        start=True,
                        stop=True,
                    )
                    nc.scalar.copy(out=seg(x0g, s), in_=pt[:])

            # elementwise schedule, two so-groups
            build_i = [0]

            def build_engine():
                # route a fraction of product builds to GpSimd
                build_i[0] += 1
                if pool_frac > 0 and (build_i[0] % 100) < pool_frac * 100:
                    return nc.gpsimd
                return nc.vector

            for sos, ops in groups:
                live = {}
                # last path index per so within this group (for stop flag)
                path_ops = [op for op in ops if op[0] == "path"]
                last_for_so = {}
                first_for_so = {}
                for i, op in enumerate(path_ops):
                    so = op[5]
                    if so not in first_for_so:
                        first_for_so[so] = i
                    last_for_so[so] = i

                acc = {}  # so -> psum tile
                with tc.tile_pool(
                    name=f"acc{sos[0]}", bufs=8, space="PSUM"
                ) as acc_pool:
                    for so in sos:
                        if any(op[5] == so for op in path_ops):
                            acc[so] = acc_pool.tile(
                                [U, ZS], mybir.dt.float32, tag=f"acc{so % 8}",
                                name=f"acc_{so}", bufs=1,
                            )

                    def ref(r):
                        kind, key = r
                        if kind == "x1":
                            return seg(x1t, key)
                        return live[r]

                    pi = 0
                    n_act = [0]
                    n_path = [0]
                    for op in ops:
                        if op[0] == "sg":
                            s0, s = op[1]
                            t = sg_pool.tile([U, ZS], dt, tag="sg")
                            live[("sg", (s0, s))] = t
                            build_engine().tensor_tensor(
                                out=t[:], in0=seg(x0g, s0), in1=seg(x1t, s), op=MULT
                            )
                        elif op[0] == "pair":
                            a, b = op[1]
                            t = pair_pool.tile([U, ZS], dt, tag="pair")
                            live[("pair", (a, b))] = t
                            build_engine().tensor_tensor(
                                out=t[:], in0=seg(x1t, a), in1=seg(x1t, b), op=MULT
                            )
                        else:
                            _, kind, r1, c, r2, so = op
                            t = tmp_pool.tile([U, ZS], dt, tag="tmp")
                            if kind == "d1":
                                # term = c * sg  -> do on ScalarE (reads sg)
                                nc.scalar.mul(t[:], ref(r1), c)
                                n_act[0] += 1
                            else:
                                n_path[0] += 1
                                use_act = (n_path[0] % 10) < act_frac * 10
                                if use_act:
                                    t1 = tmp_pool.tile([U, ZS], dt, tag="tmp")
                                    nc.vector.tensor_tensor(
                                        out=t1[:], in0=ref(r1), in1=ref(r2), op=MULT
                                    )
                                    nc.scalar.mul(t[:], t1[:], c)
                                else:
                                    nc.vector.scalar_tensor_tensor(
                                        out=t[:],
                                        in0=ref(r1),
                                        scalar=c,
                                        in1=ref(r2),
                                        op0=MULT,
                                        op1=MULT,
                                    )
                            nc.tensor.matmul(
                                acc[so][:],
                                id_sb[:],
                                t[:],
                                start=(pi == first_for_so[so]),
                                stop=(pi == last_for_so[so]),
                            )
                            pi += 1

                    # evacuate this group's accumulators
                    for so in sos:
                        if so in acc:
                            nc.scalar.copy(out=seg(out_sb, so), in_=acc[so][:])
                        else:
                            nc.vector.memset(seg(out_sb, so), 0.0)

            for s in range(S):
                nc.sync.dma_start(out=out_d[s * U : (s + 1) * U, :], in_=seg(out_sb, s))

    nc.compile()
    return nc


def kernel(x0, x1, coeff1, coeff2, coeff3, i0, idx1, idx2, idx3):
    global LAST_EXEC_NS, LAST_RESULTS
    from concourse.bass_utils import run_bass_kernel_spmd

    x0 = np.asarray(x0, dtype=np.float32)
    x1 = np.asarray(x1, dtype=np.float32)
    i0 = np.asarray(i0).astype(np.int64)
    idxs = [np.asarray(a) for a in (idx1, idx2, idx3)]
    coeffs = [np.asarray(c, dtype=np.float32) for c in (coeff1, coeff2, coeff3)]

    dtype_name = os.environ.get("KERNEL_DTYPE", "bfloat16")
    pool_frac = float(os.environ.get("KERNEL_POOL_FRAC", "0.0"))
    act_frac = float(os.environ.get("KERNEL_ACT_FRAC", "0.6"))
    npdt = np.float32
    if dtype_name != F32:
        import ml_dtypes

        npdt = ml_dtypes.bfloat16

    groups = _build_plan(idxs, coeffs)
    nc = _build_bass(groups, dtype_name, pool_frac, act_frac)

    # host-side sharding + layout transforms
    in_maps = []
    eye = np.arange(NELEM)
    ident = np.eye(U, dtype=npdt)
    x0c = x0.astype(npdt)
    for c in range(NCORES):
        zl, zh = c * ZS, (c + 1) * ZS
        shard = x1[zl:zh]  # [ZS, S*U]
        x1t = np.ascontiguousarray(
            shard.reshape(ZS, S, U).transpose(1, 2, 0).reshape(S * U, ZS)
        ).astype(npdt)
        oh = (i0[zl:zh][None, :] == eye[:, None]).astype(npdt)
        in_maps.append({"x1t": x1t, "x0w": x0c, "oh": oh, "ident": ident})

    trace = os.environ.get("BASS_TRACE", "") not in ("", "0")
    res = run_bass_kernel_spmd(nc, in_maps, core_ids=list(range(NCORES)), trace=trace)
    LAST_EXEC_NS = res.exec_time_ns
    LAST_RESULTS = res

    out = np.empty((Z, S * U), dtype=np.float32)
    for c in range(NCORES):
        outt = np.asarray(res.results[c]["outt"], dtype=np.float32)
        out[c * ZS : (c + 1) * ZS] = (
            outt.reshape(S, U, ZS).transpose(2, 0, 1).reshape(ZS, S * U)
        )
    return out


# revision 10
# speedup vs baseline: 1.6447x; 1.2095x over previous
"""Trainium2 Bass kernel for a segmented tensor-product contraction.

Computation (per batch row z, channel u, segments of width U=128):
  out[z, so, u] += c_p * x0[i0[z], s0_p, u] * prod_k x1[z, sk_p, u]
for 256 paths of degree 1..3 over S=16 segments.

Strategy:
  - Data-parallel over z across 8 NeuronCores (512 rows each).
  - On-chip layout: [u (partitions) x z (free dim)] per segment; every
    elementwise op is a [128, 512] instruction.
  - x0 row gather: host builds one-hot(i0) per core; TensorEngine computes
    x0gT[s] = x0[:, s]^T @ onehot (gather + transpose for free).
  - Factorization (globally optimized per so-group): suffix products
    sg(s0,s) = x0g[s0]*x1[s] and pairs pr(a,b) = x1[a]*x1[b]; each path is
    one tensor_tensor plus a coefficient scale on ScalarE (or a fused
    scalar_tensor_tensor on VectorE for a fraction of paths).
  - Product builds are packed into merged multi-segment instructions
    (sg runs share one instr via a stride-0 broadcast of x0g[s0]; pair
    runs along constant delta read contiguous x1 spans).
  - Output accumulation on TensorEngine: identity-matmul each path term
    into a per-segment PSUM bank (exact f32 adds). 16 output segments ->
    two groups of 8 banks; the so-partition is optimized to minimize
    duplicated product builds.
"""

import os
from collections import defaultdict

import numpy as np

U = 128
S = 16
NELEM = 64
Z = 4096
NCORES = 8
ZS = Z // NCORES  # 512 rows per core

LAST_EXEC_NS = None
LAST_RESULTS = None

F32 = "float32"


def _parse_paths(idxs, coeffs):
    paths = []  # (degree, x1segs_sorted, s0, so, coeff)
    for idx, cf in zip(idxs, coeffs):
        d = idx.shape[1] - 2
        for r, c in zip(idx, cf):
            r = [int(v) for v in r]
            paths.append((d, tuple(sorted(r[:d])), r[d], r[d + 1], float(c)))
    return paths


def _options(p):
    """Candidate (products, form) decompositions for a path.

    Each option: (frozenset of product keys, form)
    form = (in0_ref, in1_ref) with refs ('x1',s) ('x0g',s) ('sg',(s0,s))
    ('pair',(a,b)); d1 form = (('sg',(s0,s)), None).
    Product keys: ('sg',(s0,s)), ('pair',(a,b)).
    """
    d, segs, s0, so, c = p
    if d == 1:
        k = ("sg", (s0, segs[0]))
        return [(frozenset([k]), (k, None))]
    if d == 2:
        a, b = segs
        opts = [
            (frozenset([("sg", (s0, b))]), (("x1", a), ("sg", (s0, b)))),
            (frozenset([("sg", (s0, a))]), (("x1", b), ("sg", (s0, a)))),
            (frozenset([("pair", (a, b))]), (("pair", (a, b)), ("x0g", s0))),
        ]
        return opts
    a, b, cc = segs
    return [
        (
            frozenset([("pair", (a, b)), ("sg", (s0, cc))]),
            (("pair", (a, b)), ("sg", (s0, cc))),
        ),
        (
            frozenset([("pair", (a, cc)), ("sg", (s0, b))]),
            (("pair", (a, cc)), ("sg", (s0, b))),
        ),
        (
            frozenset([("pair", (b, cc)), ("sg", (s0, a))]),
            (("pair", (b, cc)), ("sg", (s0, a))),
        ),
    ]


def _optimize_group(gpaths, n_sweeps=4):
    """Choose per-path decomposition minimizing total unique products."""
    choices = [0] * len(gpaths)
    opts = [_options(p) for p in gpaths]
    for _ in range(n_sweeps):
        counts = defaultdict(int)
        for i, p in enumerate(gpaths):
            for k in opts[i][choices[i]][0]:
                counts[k] += 1
        changed = False
        for i, p in enumerate(gpaths):
            best, best_cost = choices[i], None
            for j, (prods, _) in enumerate(opts[i]):
                # marginal cost: products not used by anyone else
                cost = 0.0
                for k in prods:
                    others = counts[k] - (1 if k in opts[i][choices[i]][0] else 0)
                    cost += 1.0 / (1 + others)
                if best_cost is None or cost < best_cost - 1e-9:
                    best, best_cost = j, cost
            if best != choices[i]:
                # update counts incrementally
                for k in opts[i][choices[i]][0]:
                    counts[k] -= 1
                for k in opts[i][best][0]:
                    counts[k] += 1
                choices[i] = best
                changed = True
        if not changed:
            break
    products = set()
    forms = []
    for i, p in enumerate(gpaths):
        prods, form = opts[i][choices[i]]
        products |= prods
        forms.append(form)
    return products, forms


def _group_cost(paths, sos_a):
    """Estimate total builds for a candidate so-partition."""
    total = 0
    for sos in (sos_a, [s for s in range(S) if s not in sos_a]):
        gp = [p for p in paths if p[3] in sos]
        prods, _ = _optimize_group(gp, n_sweeps=2)
        total += len(prods)
    return total


def _optimize_partition(paths):
    """Greedy swap optimization of the 8/8 so-partition."""
    cur = list(range(8))
    cur_cost = _group_cost(paths, cur)
    import random

    rng = random.Random(0)
    for _ in range(40):
        a = rng.choice(cur)
        b = rng.choice([s for s in range(S) if s not in cur])
        cand = sorted([s for s in cur if s != a] + [b])
        cost = _group_cost(paths, cand)
        if cost < cur_cost:
            cur, cur_cost = cand, cost
    other = [s for s in range(S) if s not in cur]
    return cur, other


def _plan_merges(products):
    """Pack product builds into merged instructions.

    Returns (slot_of, builds) where slot_of maps product key -> slot index
    and builds is a list of ('sg_run', s0, s_lo, n, slot_lo) or
    ('pair_run', delta, a_lo, n, slot_lo).
    """
    slot_of = {}
    builds = []
    next_slot = 0
    sgs = defaultdict(list)  # s0 -> sorted s list
    prs = defaultdict(list)  # delta -> sorted a list
    for k in products:
        if k[0] == "sg":
            sgs[k[1][0]].append(k[1][1])
        else:
            a, b = k[1]
            prs[b - a].append(a)
    for s0 in sorted(sgs):
        ss = sorted(sgs[s0])
        run = [ss[0]]
        for s in ss[1:] + [None]:
            if s is not None and s == run[-1] + 1:
                run.append(s)
            else:
                builds.append(("sg_run", s0, run[0], len(run), next_slot))
                for i, rs in enumerate(run):
                    slot_of[("sg", (s0, rs))] = next_slot + i
                next_slot += len(run)
                if s is not None:
                    run = [s]
    for delta in sorted(prs):
        aa = sorted(prs[delta])
        run = [aa[0]]
        for a in aa[1:] + [None]:
            if a is not None and a == run[-1] + 1:
                run.append(a)
            else:
                builds.append(("pair_run", delta, run[0], len(run), next_slot))
                for i, ra in enumerate(run):
                    slot_of[("pair", (ra, ra + delta))] = next_slot + i
                next_slot += len(run)
                if a is not None:
                    run = [a]
    return slot_of, builds, next_slot


def _build_plan(idxs, coeffs):
    """Full schedule. Returns list of group dicts."""
    paths = _parse_paths(idxs, coeffs)
    part_a, part_b = _optimize_partition(paths)
    groups = []
    for sos in (part_a, part_b):
        gp = [p for p in paths if p[3] in sos]
        products, forms = _optimize_group(gp)
        slot_of, builds, n_slots = _plan_merges(products)
        # order paths: d1 first (feed ACT early), then by first product slot
        order = sorted(
            range(len(gp)),
            key=lambda i: (
                gp[i][0] != 1,
                max(
                    (slot_of[r] for r in forms[i] if r and r[0] in ("sg", "pair")),
                    default=-1,
                ),
            ),
        )
        path_ops = [
            (gp[i][0], forms[i][0], forms[i][1], gp[i][4], gp[i][3]) for i in order
        ]
        groups.append(
            dict(
                sos=sos,
                builds=builds,
                slot_of=slot_of,
                n_slots=n_slots,
                path_ops=path_ops,
            )
        )
    return groups


def _build_bass(groups, dtype_name, act_frac, warmup):
    import concourse.bacc as bacc
    import concourse.mybir as mybir
    from concourse.tile import TileContext

    dt = mybir.dt.float32 if dtype_name == F32 else mybir.dt.bfloat16
    MULT = mybir.AluOpType.mult

    nc = bacc.Bacc("TRN2", debug=False)

    x1t_d = nc.dram_tensor("x1t", [S * U, ZS], dt, kind="ExternalInput")
    x0_d = nc.dram_tensor("x0w", [NELEM, S * U], dt, kind="ExternalInput")
    oh_d = nc.dram_tensor("oh", [NELEM, ZS], dt, kind="ExternalInput")
    id_d = nc.dram_tensor("ident", [U, U], dt, kind="ExternalInput")
    out_d = nc.dram_tensor("outt", [S * U, ZS], dt, kind="ExternalOutput")
    junk_d = nc.dram_tensor("junk", [U, ZS], mybir.dt.float32)

    max_slots = max(g["n_slots"] for g in groups)

    with TileContext(nc) as tc:
        with tc.tile_pool(name="persist", bufs=1) as persist, tc.tile_pool(
            name="tmp", bufs=12
        ) as tmp_pool:
            x1t = persist.tile([U, S * ZS], dt, tag="x1t")
            x0g = persist.tile([U, S * ZS], dt, tag="x0g")
            out_sb = persist.tile([U, S * ZS], dt, tag="out")
            prod = persist.tile([U, max_slots * ZS], dt, tag="prod")
            x0_sb = persist.tile([NELEM, S * U], dt, tag="x0w")
            oh_sb = persist.tile([NELEM, ZS], dt, tag="oh")
            id_sb = persist.tile([U, U], dt, tag="ident")

            def seg(t, s):
                return t[:, s * ZS : (s + 1) * ZS]

            def span(t, lo, n):
                return t[:, lo * ZS : (lo + n) * ZS]

            for s in range(S):
                nc.sync.dma_start(out=seg(x1t, s), in_=x1t_d[s * U : (s + 1) * U, :])
            nc.sync.dma_start(out=x0_sb[:], in_=x0_d[:])
            nc.sync.dma_start(out=oh_sb[:], in_=oh_d[:])
            nc.sync.dma_start(out=id_sb[:], in_=id_d[:])

            # PE warmup burst + gather matmuls
            with tc.tile_pool(name="gpsum", bufs=4, space="PSUM") as gpsum:
                if warmup > 0:
                    wt = gpsum.tile([U, ZS], mybir.dt.float32, tag="warm", bufs=1)
                    for i in range(warmup):
                        nc.tensor.matmul(
                            wt[:],
                            x0_sb[:, 0:U],
                            oh_sb[:],
                            start=(i == 0),
                            stop=(i == warmup - 1),
                        )
                    ws = tmp_pool.tile([U, ZS], mybir.dt.float32, tag="warms")
                    nc.scalar.copy(out=ws[:], in_=wt[:])
                    nc.sync.dma_start(out=junk_d[:], in_=ws[:])
                for s in range(S):
                    pt = gpsum.tile([U, ZS], mybir.dt.float32, tag="gps")
                    nc.tensor.matmul(
                        pt[:],
                        x0_sb[:, s * U : (s + 1) * U],
                        oh_sb[:],
                        start=True,
                        stop=True,
                    )
                    nc.scalar.copy(out=seg(x0g, s), in_=pt[:])

            for g in groups:
                sos, builds, slot_of, path_ops = (
                    g["sos"],
                    g["builds"],
                    g["slot_of"],
                    g["path_ops"],
                )
                # interleave: emit builds, releasing paths when ready
                ready_after = defaultdict(list)  # build idx -> path indices
                path_needs = []
                for i, (d, r1, r2, c, so) in enumerate(path_ops):
                    needs = set()
                    for r in (r1, r2):
                        if r and r[0] in ("sg", "pair"):
                            needs.add(slot_of[r])
                    path_needs.append(needs)
                slot_done_at = {}
                for bi, b in enumerate(builds):
                    for i in range(b[3]):
                        slot_done_at[b[4] + i] = bi
                for i, needs in enumerate(path_needs):
                    bi = max((slot_done_at[s] for s in needs), default=-1)
                    ready_after[bi].append(i)

                # emission order determines PE program order: derive
                # first/last per so from it for the start/stop flags
                emit_order = list(ready_after[-1])
                for bi in range(len(builds)):
                    emit_order.extend(ready_after[bi])
                first_for_so = {}
                last_for_so = {}
                for i in emit_order:
                    so = path_ops[i][4]
                    if so not in first_for_so:
                        first_for_so[so] = i
                    last_for_so[so] = i

                acc = {}
                with tc.tile_pool(
                    name=f"acc{sos[0]}", bufs=8, space="PSUM"
                ) as acc_pool:
                    for so in sos:
                        if so in first_for_so:
                            acc[so] = acc_pool.tile(
                                [U, ZS],
                                mybir.dt.float32,
                                tag=f"acc{sos.index(so)}",
                                name=f"acc_{so}",
                                bufs=1,
                            )

                    def pref(r):
                        kind, key = r
                        if kind == "x1":
                            return seg(x1t, key)
                        if kind == "x0g":
                            return seg(x0g, key)
                        sl = slot_of[r]
                        return seg(prod, sl)

                    n_path = [0]

                    def emit_path(i):
                        d, r1, r2, c, so = path_ops[i]
                        t = tmp_pool.tile([U, ZS], dt, tag="tmp", name=f"t{i}")
                        if d == 1:
                            nc.scalar.mul(t[:], pref(r1), c)
                        else:
                            n_path[0] += 1
                            if (n_path[0] % 10) < act_frac * 10:
                                t1 = tmp_pool.tile(
                                    [U, ZS], dt, tag="tmp", name=f"t1{i}"
                                )
                                nc.vector.tensor_tensor(
                                    out=t1[:], in0=pref(r1), in1=pref(r2), op=MULT
                                )
                                nc.scalar.mul(t[:], t1[:], c)
                            else:
                                nc.vector.scalar_tensor_tensor(
                                    out=t[:],
                                    in0=pref(r1),
                                    scalar=c,
                                    in1=pref(r2),
                                    op0=MULT,
                                    op1=MULT,
                                )
                        pi = i  # position among path_ops
                        nc.tensor.matmul(
                            acc[so][:],
                            id_sb[:],
                            t[:],
                            start=(pi == first_for_so[so]),
                            stop=(pi == last_for_so[so]),
                        )

                    for i in ready_after[-1]:
                        emit_path(i)
                    for bi, b in enumerate(builds):
                        kind = b[0]
                        if kind == "sg_run":
                            _, s0, s_lo, n, slot_lo = b
                            in0 = (
                                seg(x0g, s0)
                                .rearrange("p (o z) -> p o z", o=1)
                                .broadcast_to([U, n, ZS])
                            )
                            in1 = span(x1t, s_lo, n).rearrange(
                                "p (r z) -> p r z", r=n
                            )
                            out = span(prod, slot_lo, n).rearrange(
                                "p (r z) -> p r z", r=n
                            )
                            nc.vector.tensor_tensor(
                                out=out, in0=in0, in1=in1, op=MULT
                            )
                        else:
                            _, delta, a_lo, n, slot_lo = b
                            in0 = span(x1t, a_lo, n).rearrange(
                                "p (r z) -> p r z", r=n
                            )
                            in1 = span(x1t, a_lo + delta, n).rearrange(
                                "p (r z) -> p r z", r=n
                            )
                            out = span(prod, slot_lo, n).rearrange(
                                "p (r z) -> p r z", r=n
                            )
                            nc.vector.tensor_tensor(
                                out=out, in0=in0, in1=in1, op=MULT
                            )
                        for i in ready_after[bi]:
                            emit_path(i)

                    for so in sos:
                        if so in acc:
                            nc.scalar.copy(out=seg(out_sb, so), in_=acc[so][:])
                        else:
                            nc.vector.memset(seg(out_sb, so), 0.0)

            for s in range(S):
                nc.sync.dma_start(out=out_d[s * U : (s + 1) * U, :], in_=seg(out_sb, s))

    nc.compile()
    return nc


def kernel(x0, x1, coeff1, coeff2, coeff3, i0, idx1, idx2, idx3):
    global LAST_EXEC_NS, LAST_RESULTS
    from concourse.bass_utils import run_bass_kernel_spmd

    x0 = np.asarray(x0, dtype=np.float32)
    x1 = np.asarray(x1, dtype=np.float32)
    i0 = np.asarray(i0).astype(np.int64)
    idxs = [np.asarray(a) for a in (idx1, idx2, idx3)]
    coeffs = [np.asarray(c, dtype=np.float32) for c in (coeff1, coeff2, coeff3)]

    dtype_name = os.environ.get("KERNEL_DTYPE", "bfloat16")
    act_frac = float(os.environ.get("KERNEL_ACT_FRAC", "0.9"))
    warmup = int(os.environ.get("KERNEL_WARMUP", "16"))
    npdt = np.float32
    if dtype_name != F32:
        import ml_dtypes

        npdt = ml_dtypes.bfloat16

    groups = _build_plan(idxs, coeffs)
    nc = _build_bass(groups, dtype_name, act_frac, warmup)

    in_maps = []
    eye = np.arange(NELEM)
    ident = np.eye(U, dtype=npdt)
    x0c = x0.astype(npdt)
    for c in range(NCORES):
        zl, zh = c * ZS, (c + 1) * ZS
        shard = x1[zl:zh]
        x1t = np.ascontiguousarray(
            shard.reshape(ZS, S, U).transpose(1, 2, 0).reshape(S * U, ZS)
        ).astype(npdt)
        oh = (i0[zl:zh][None, :] == eye[:, None]).astype(npdt)
        in_maps.append({"x1t": x1t, "x0w": x0c, "oh": oh, "ident": ident})

    trace = os.environ.get("BASS_TRACE", "") not in ("", "0")
    res = run_bass_kernel_spmd(nc, in_maps, core_ids=list(range(NCORES)), trace=trace)
    LAST_EXEC_NS = res.exec_time_ns
    LAST_RESULTS = res

    out = np.empty((Z, S * U), dtype=np.float32)
    for c in range(NCORES):
        outt = np.asarray(res.results[c]["outt"], dtype=np.float32)
        out[c * ZS : (c + 1) * ZS] = (
            outt.reshape(S, U, ZS).transpose(2, 0, 1).reshape(ZS, S * U)
        )
    return out
